# revision 17
# baseline (speedup 1.0000x reference)
import sys

sys.path.insert(0, "/opt/trn_rl_repo")
from contextlib import ExitStack

import numpy as np

import concourse.bass as bass  # noqa: F401
import concourse.mybir as mybir
import concourse.tile as tile
from concourse import bacc
from concourse.ap import AP
from concourse.bass_utils import run_bass_kernel_spmd

F32 = mybir.dt.float32
F32R = mybir.dt.float32r
BF16 = mybir.dt.bfloat16
FP8 = mybir.dt.float8e4
AF = mybir.ActivationFunctionType
ALU = mybir.AluOpType
AX = mybir.AxisListType
DR = mybir.MatmulPerfMode.DoubleRow
EPS = 1e-5
WS = 64.0  # weight pre-scale so fp8 hi/lo residuals stay in range
IWS = 1.0 / WS

NP_BF16 = mybir.dt.np(BF16)
NP_FP8 = mybir.dt.np(FP8)


def fr(ap):
    return ap.bitcast(F32R)


def mk(t, fs, base, n, off, dims):
    """Manual AP on tile t: partitions [base, base+n), free offset off, dims."""
    return AP(t[:].tensor, base * fs + off, [[fs, n]] + [list(d) for d in dims])


def build_program():
    nc = bacc.Bacc(trn_type="TRN2")

    def din(name, shape, dt_=F32):
        return nc.dram_tensor(name, shape, dt_, kind="ExternalInput")

    xs = din("xs", [8, 3, 224, 224], FP8)
    stem_l = din("stem_l", [128, 768], FP8)
    w1e_d = din("w1e", [128, 576], BF16)
    w2ea_d = din("w2ea", [128, 1152], BF16)
    w2eb_d = din("w2eb", [128, 1152], BF16)
    bid1_d = din("bid1", [128, 32], BF16)
    bid2_d = din("bid2", [128, 64], BF16)
    e1x_d = din("e1x", [4, 128], F32R)
    e2a_d = din("e2a", [4, 128], F32R)
    e2b_d = din("e2b", [4, 128], F32R)
    r1l_d = din("r1l", [32, 4], F32R)
    r1b_d = din("r1b", [4, 1])
    r2l_d = din("r2l", [64, 4], F32R)
    r2b_d = din("r2b", [4, 1])
    c1bT_d = din("c1bT", [4, 64], F32R)
    bn1h_d = din("bn1h64", [64, 1])
    c2bT_d = din("c2bT", [4, 128], F32R)
    bn2h_d = din("bn2h", [128, 1])
    bn0h_d = din("bn0h", [128, 1])
    fcl_d = din("fcl", [128, 2], F32R)
    fcb_d = din("fcb", [2, 1])
    out_d = nc.dram_tensor("out", [2, 8], F32, kind="ExternalOutput")

    with tile.TileContext(nc) as tc, ExitStack() as ctx:
        def P(name, bufs, space="SBUF"):
            return ctx.enter_context(tc.tile_pool(name=name, bufs=bufs, space=space))

        consts = P("consts", 1)
        xrp = P("xr", 1)
        h0p = P("h0", 2)
        h1p = P("h1", 2)
        cw1p = P("cw1", 2)
        cw2p = P("cw2", 2)
        sc1p = P("sc1", 2)
        sc2p = P("sc2", 2)
        scrp = P("scr", 2)
        smallp = P("small", 4)
        gapp = P("gap", 2)
        ps = P("ps", 4, "PSUM")

        def lc(dt_, shape, tag, tdt=F32):
            t = consts.tile(shape, tdt, tag=tag)
            # consts on the scalar DGE queue: interleaving small f32 DMAs with
            # the strided 8-bit xs loads on one queue corrupts transfers
            nc.scalar.dma_start(t[:], dt_[:, :])
            return t

        stem_sb = lc(stem_l, [128, 768], "stem_l", FP8)
        w1e = lc(w1e_d, [128, 576], "w1e", BF16)
        w2ea = lc(w2ea_d, [128, 1152], "w2ea", BF16)
        w2eb = lc(w2eb_d, [128, 1152], "w2eb", BF16)
        bid1 = lc(bid1_d, [128, 32], "bid1", BF16)
        bid2 = lc(bid2_d, [128, 64], "bid2", BF16)
        e1x = lc(e1x_d, [4, 128], "e1x", F32R)
        e2a = lc(e2a_d, [4, 128], "e2a", F32R)
        e2b = lc(e2b_d, [4, 128], "e2b", F32R)
        r1l = lc(r1l_d, [32, 4], "r1l", F32R)
        r1b = lc(r1b_d, [4, 1], "r1b")
        r2l = lc(r2l_d, [64, 4], "r2l", F32R)
        r2b = lc(r2b_d, [4, 1], "r2b")
        c1bT = lc(c1bT_d, [4, 64], "c1bT", F32R)
        bn1h = lc(bn1h_d, [64, 1], "bn1h")
        c2bT = lc(c2bT_d, [4, 128], "c2bT", F32R)
        bn2h = lc(bn2h_d, [128, 1], "bn2h")
        bn0h = lc(bn0h_d, [128, 1], "bn0h")
        fcl = lc(fcl_d, [128, 2], "fcl", F32R)
        fcb = lc(fcb_d, [2, 1], "fcb")

        # ---- x preload: xr[64g + 9s + 3ky + c, R, C] = x[4g+s, c, 2R-1+ky, C-2]
        xr = xrp.tile([100, 112 * 228], FP8, tag="xr")
        xr3 = xr[:].rearrange("p (r c) -> p r c", c=228)
        nc.gpsimd.memset(xr3[0:100, :, 0:2], 0.0)
        nc.gpsimd.memset(xr3[0:100, 0:1, :], 0.0)
        for g in range(2):
            for s in range(4):
                base = 64 * g + 9 * s
                nc.sync.dma_start(xr3[base : base + 3, 1:112, 2:226],
                                  xs[4 * g + s, :, 1:222:2, :])
                nc.sync.dma_start(xr3[base + 3 : base + 6, 0:112, 2:226],
                                  xs[4 * g + s, :, 0:223:2, :])
                nc.sync.dma_start(xr3[base + 6 : base + 9, 0:112, 2:226],
                                  xs[4 * g + s, :, 1:224:2, :])

        pooled1 = smallp.tile([32, 8], F32R, tag="pooled1")
        pooled2 = smallp.tile([64, 8], F32R, tag="pooled2")
        pooled3 = smallp.tile([128, 8], F32R, tag="pooled3")

        h0s, gap0s = [], []
        # ================= stems =================
        for g in range(2):
            h0 = h0p.tile([128, 114 * 114], FP8, tag="h0")
            h03 = h0[:].rearrange("p (r c) -> p r c", c=114)
            nc.gpsimd.memset(h03[:, 0:1, :], 0.0)
            nc.gpsimd.memset(h03[:, 113:114, :], 0.0)
            nc.gpsimd.memset(h03[:, :, 0:1], 0.0)
            nc.gpsimd.memset(h03[:, :, 113:114], 0.0)
            h0s.append(h0)
            gap0 = gapp.tile([128, 14], F32, tag="gap0")
            gap0s.append(gap0)
            for sy in range(14):
                pst = ps.tile([128, 1024], F32, tag="pb")
                for c2 in range(2):
                    y0 = 8 * sy + 4 * c2
                    for kx in range(3):
                        nc.tensor.matmul(
                            pst[:, 512 * c2 : 512 * c2 + 448],
                            mk(stem_sb, 768, 64 * g, 36, 256 * kx,
                               [(128, 2), (1, 128)]),
                            mk(xr, 112 * 228, 64 * g, 36, y0 * 228 + kx + 1,
                               [(0, 2), (228, 4), (2, 112)]),
                            start=(kx == 0), stop=(kx == 2), perf_mode=DR,
                        )
                nc.scalar.activation(
                    h03[:, 8 * sy + 1 : 8 * sy + 9, 1:113].rearrange(
                        "p (a b) c -> p a b c", a=2),
                    pst[:, 0:1024].rearrange("p (a b) -> p a b", a=2)[:, :, 0:448]
                    .rearrange("p a (b c) -> p a b c", b=4),
                    AF.Relu, bias=bn0h[:, 0:1], scale=IWS,
                    accum_out=gap0[:, sy : sy + 1],
                )

        # ========== routing1 + conv1 weight combine (per group) ==========
        cwts, bias1s = [], []
        for g in range(2):
            g1 = smallp.tile([128, 1], F32, tag="g1")
            nc.vector.tensor_reduce(g1[:], gap0s[g][:], AX.X, ALU.add)
            for s in range(4):
                nc.scalar.dma_start(pooled1[:, 4 * g + s : 4 * g + s + 1],
                                    g1[32 * s : 32 * s + 32, :].bitcast(F32R))
            psr = ps.tile([128, 1024], F32, tag="pb")
            nc.tensor.matmul(psr[0:4, 0:4], fr(r1l[:]),
                             fr(pooled1[:, 4 * g : 4 * g + 4]), start=True, stop=True)
            r1g = smallp.tile([4, 4], F32R, tag="r1g")
            nc.scalar.activation(r1g[:], psr[0:4, 0:4], AF.Sigmoid,
                                 bias=r1b[:, 0:1], scale=1.0)
            psb = ps.tile([128, 1024], F32, tag="pb")
            nc.tensor.matmul(psb[:, 0:4], fr(e1x[:]), fr(r1g[:]), start=True, stop=True)
            r1full = smallp.tile([128, 4], F32, tag="r1full")
            nc.scalar.copy(r1full[:], psb[:, 0:4])
            psc = ps.tile([128, 1024], F32, tag="pb")
            nc.tensor.matmul(psc[0:64, 0:4], fr(c1bT[:]), fr(r1g[:]), start=True, stop=True)
            bias1 = smallp.tile([64, 4], F32, tag="bias1")
            nc.scalar.activation(bias1[:], psc[0:64, 0:4], AF.Identity,
                                 bias=bn1h[:, 0:1], scale=1.0)
            bias1s.append(bias1)
            # cwt[64pr+32sl+c, 256t + 128tau + cc] block-diag in cc = 64sl+co
            cwt = cw1p.tile([128, 2304], FP8, tag="cwt")
            cwts.append(cwt)
            nc.gpsimd.memset(cwt[:, :], 0.0)
            psw = ps.tile([128, 1024], F32, tag="pb")
            for j in range(4):
                tj = sc1p.tile([128, 576], BF16, tag="sc1")
                nc.vector.tensor_scalar(tj[:], w1e[:], r1full[:, j : j + 1],
                                        None, ALU.mult)
                for h in range(2):
                    nc.tensor.matmul(
                        psw[32 * j : 32 * j + 32, 512 * h : 512 * h + 288],
                        bid1[:], tj[:, 288 * h : 288 * h + 288],
                        start=True, stop=True, tile_position=(0, 32 * j),
                    )
            for pr in range(2):
                for sl in range(2):
                    j = 2 * pr + sl
                    pin = psw[32 * j : 32 * j + 32, 0:1024].rearrange(
                        "p (a b) -> p a b", a=2)[:, :, 0:288].rearrange(
                        "p a (co t) -> p a co t", t=9)
                    hi = mk(cwt, 2304, 64 * pr + 32 * sl, 32, 64 * sl,
                            [(32, 2), (1, 32), (256, 9)])
                    lo = mk(cwt, 2304, 64 * pr + 32 * sl, 32, 64 * sl + 128,
                            [(32, 2), (1, 32), (256, 9)])
                    nc.scalar.copy(hi, pin)
                    nc.vector.tensor_tensor(lo, pin, hi, ALU.subtract)

        # ===== conv1 / routing2 / conv2, software-pipelined across pairs =====
        h1s = {}
        gap1s = {}

        def conv1(p):
            g, pr = divmod(p, 2)
            h1 = h1p.tile([128, 114 * 114], FP8, tag="h1")
            h13 = h1[:].rearrange("p (r c) -> p r c", c=114)
            nc.gpsimd.memset(h13[:, 0:1, :], 0.0)
            nc.gpsimd.memset(h13[:, 113:114, :], 0.0)
            nc.gpsimd.memset(h13[:, :, 0:1], 0.0)
            nc.gpsimd.memset(h13[:, :, 113:114], 0.0)
            h1s[p] = h1
            pbias = smallp.tile([128, 1], F32, tag="pbias")
            nc.scalar.dma_start(pbias[0:64, :], bias1s[g][:, 2 * pr : 2 * pr + 1])
            nc.scalar.dma_start(pbias[64:128, :], bias1s[g][:, 2 * pr + 1 : 2 * pr + 2])
            gap1 = gapp.tile([128, 14], F32, tag="gap1")
            gap1s[p] = gap1
            cwt, h0 = cwts[g], h0s[g]
            for grp in range(14):
                pst = ps.tile([128, 1024], F32, tag="pb")
                for c2 in range(2):
                    y0 = 8 * grp + 4 * c2
                    for t in range(9):
                        ky, kx = t // 3, t % 3
                        nc.tensor.matmul(
                            pst[:, 512 * c2 : 512 * c2 + 448],
                            mk(cwt, 2304, 64 * pr, 64, 256 * t, [(128, 2), (1, 128)]),
                            mk(h0, 114 * 114, 64 * pr, 64,
                               (y0 + ky) * 114 + kx, [(0, 2), (114, 4), (1, 112)]),
                            start=(t == 0), stop=(t == 8), perf_mode=DR,
                        )
                nc.scalar.activation(
                    h13[:, 8 * grp + 1 : 8 * grp + 9, 1:113].rearrange(
                        "p (a b) c -> p a b c", a=2),
                    pst[:, 0:1024].rearrange("p (a b) -> p a b", a=2)[:, :, 0:448]
                    .rearrange("p a (b c) -> p a b c", b=4),
                    AF.Relu, bias=pbias[:, 0:1], scale=IWS,
                    accum_out=gap1[:, grp : grp + 1],
                )

        def r2c2(p):
            g, pr = divmod(p, 2)
            h1 = h1s[p]
            g2 = smallp.tile([128, 1], F32, tag="g2")
            nc.vector.tensor_reduce(g2[:], gap1s[p][:], AX.X, ALU.add)
            col0 = 2 * p
            nc.scalar.dma_start(pooled2[:, col0 : col0 + 1], g2[0:64, :].bitcast(F32R))
            nc.scalar.dma_start(pooled2[:, col0 + 1 : col0 + 2],
                                g2[64:128, :].bitcast(F32R))
            ps2 = ps.tile([128, 1024], F32, tag="pb")
            nc.tensor.matmul(ps2[0:4, 0:2], fr(r2l[:]),
                             fr(pooled2[:, col0 : col0 + 2]), start=True, stop=True)
            r2g = smallp.tile([4, 2], F32R, tag="r2g")
            nc.scalar.activation(r2g[:], ps2[0:4, 0:2], AF.Sigmoid,
                                 bias=r2b[:, 0:1], scale=1.0)
            psb2 = ps.tile([128, 1024], F32, tag="pb")
            nc.tensor.matmul(psb2[:, 0:2], fr(e2a[:]), fr(r2g[:]), start=True, stop=True)
            nc.tensor.matmul(psb2[:, 512:514], fr(e2b[:]), fr(r2g[:]), start=True, stop=True)
            r2full = smallp.tile([128, 4], F32, tag="r2full")
            nc.scalar.copy(
                r2full[:].rearrange("p (a b) -> p a b", a=2),
                psb2[:, 0:1024].rearrange("p (a b) -> p a b", a=2)[:, :, 0:2],
            )
            psc2 = ps.tile([128, 1024], F32, tag="pb")
            nc.tensor.matmul(psc2[:, 0:2], fr(c2bT[:]), fr(r2g[:]), start=True, stop=True)
            bias2 = smallp.tile([128, 2], F32, tag="bias2")
            nc.scalar.activation(bias2[:], psc2[:, 0:2], AF.Identity,
                                 bias=bn2h[:, 0:1], scale=1.0)
            for sl in range(2):
                # cw2f[64sl+c, 256t + 128tau + co], co over all 128 outputs
                cw2f = cw2p.tile([128, 2304], FP8, tag="cw2f")
                ta = sc2p.tile([128, 1152], BF16, tag="sc2")
                nc.vector.tensor_scalar(ta[:], w2ea[:], r2full[:, sl : sl + 1],
                                        None, ALU.mult)
                tb = sc2p.tile([128, 1152], BF16, tag="sc2")
                nc.vector.tensor_scalar(tb[:], w2eb[:], r2full[:, 2 + sl : 3 + sl],
                                        None, ALU.mult)
                for half in range(2):
                    psw2 = ps.tile([128, 1024], F32, tag="pb")
                    for q in range(2):
                        qq = 2 * half + q
                        nc.tensor.matmul(
                            psw2[64 * sl : 64 * sl + 64, 512 * q : 512 * q + 288],
                            bid2[:], ta[:, 288 * qq : 288 * qq + 288],
                            start=True, stop=False, tile_position=(0, 64 * sl),
                        )
                        nc.tensor.matmul(
                            psw2[64 * sl : 64 * sl + 64, 512 * q : 512 * q + 288],
                            bid2[:], tb[:, 288 * qq : 288 * qq + 288],
                            start=False, stop=True, tile_position=(0, 64 * sl),
                        )
                    pin = psw2[64 * sl : 64 * sl + 64, 0:1024].rearrange(
                        "p (a b) -> p a b", a=2)[:, :, 0:288].rearrange(
                        "p a (co t) -> p a co t", t=9)
                    hi = mk(cw2f, 2304, 64 * sl, 64, 64 * half,
                            [(32, 2), (1, 32), (256, 9)])
                    lo = mk(cw2f, 2304, 64 * sl, 64, 64 * half + 128,
                            [(32, 2), (1, 32), (256, 9)])
                    nc.scalar.copy(hi, pin)
                    nc.vector.tensor_tensor(lo, pin, hi, ALU.subtract)
                gap2 = gapp.tile([128, 4], F32, tag="gap2")
                for grp in range(4):
                    nch = 2 if grp < 3 else 1
                    pst = ps.tile([128, 1024], F32, tag="pb")
                    for c2 in range(nch):
                        y0 = 8 * (2 * grp + c2)
                        for t in range(9):
                            ky, kx = t // 3, t % 3
                            nc.tensor.matmul(
                                pst[:, 512 * c2 : 512 * c2 + 448],
                                mk(cw2f, 2304, 64 * sl, 64, 256 * t,
                                   [(128, 2), (1, 128)]),
                                mk(h1, 114 * 114, 64 * sl, 64,
                                   (2 * y0 + ky) * 114 + kx,
                                   [(0, 2), (228, 8), (2, 56)]),
                                start=(t == 0), stop=(t == 8), perf_mode=DR,
                            )
                    scr = scrp.tile([128, 896], BF16, tag="scr")
                    if nch == 2:
                        nc.scalar.activation(
                            scr[:].rearrange("p (a b) -> p a b", a=2),
                            pst[:, 0:1024].rearrange("p (a b) -> p a b", a=2)
                            [:, :, 0:448],
                            AF.Relu, bias=bias2[:, sl : sl + 1], scale=IWS,
                            accum_out=gap2[:, grp : grp + 1],
                        )
                    else:
                        nc.scalar.activation(
                            scr[:, 0:448], pst[:, 0:448],
                            AF.Relu, bias=bias2[:, sl : sl + 1], scale=IWS,
                            accum_out=gap2[:, grp : grp + 1],
                        )
                g3 = smallp.tile([128, 1], F32, tag="g3")
                nc.vector.tensor_reduce(g3[:], gap2[:], AX.X, ALU.add)
                nc.scalar.dma_start(pooled3[:, 2 * p + sl : 2 * p + sl + 1],
                                    g3[:].bitcast(F32R))

        conv1(0)
        conv1(1)
        r2c2(0)
        conv1(2)
        r2c2(1)
        conv1(3)
        r2c2(2)
        r2c2(3)

        # ================= head =================
        psf = ps.tile([128, 1024], F32, tag="pb")
        nc.tensor.matmul(psf[0:2, 0:8], fr(fcl[:]), fr(pooled3[:]), start=True, stop=True)
        outsb = smallp.tile([2, 8], F32, tag="outsb")
        nc.scalar.activation(outsb[:], psf[0:2, 0:8], AF.Identity,
                             bias=fcb[:, 0:1], scale=1.0)
        nc.scalar.dma_start(out_d[:, :], outsb[:])
    nc.finalize()
    return nc


def prep_consts(i):
    def bn(g, b, m, v):
        sc = g / np.sqrt(v + EPS)
        return sc.astype(np.float32), (b - m * sc).astype(np.float32)

    def hilo(a):
        hi = a.astype(NP_FP8)
        lo = (a - hi.astype(np.float32)).astype(NP_FP8)
        return hi, lo

    c = {}
    s0, h0v = bn(*[np.asarray(i[k], np.float32) for k in ("bn0_g", "bn0_b", "bn0_m", "bn0_v")])
    s1, h1v = bn(*[np.asarray(i[k], np.float32) for k in ("bn1_g", "bn1_b", "bn1_m", "bn1_v")])
    s2, h2v = bn(*[np.asarray(i[k], np.float32) for k in ("bn2_g", "bn2_b", "bn2_m", "bn2_v")])

    # stem_l[64g + 9s + 3ky + c, 256kx + 128tau + 32s + co] = hilo(W*s0*WS)
    sw = np.asarray(i["stem_w"], np.float32) * s0[:, None, None, None] * WS
    base = sw.transpose(2, 1, 3, 0)  # [ky, c, kx, co]
    stem_f = np.zeros((128, 384), np.float32)
    for g in range(2):
        for s in range(4):
            for ky in range(3):
                for cc in range(3):
                    for kx in range(3):
                        stem_f[64 * g + 9 * s + 3 * ky + cc,
                               128 * kx + 32 * s : 128 * kx + 32 * s + 32] = base[ky, cc, kx]
    hi, lo = hilo(stem_f)
    stem_l = np.zeros((128, 768), NP_FP8)
    for kx in range(3):
        stem_l[:, 256 * kx : 256 * kx + 128] = hi[:, 128 * kx : 128 * kx + 128]
        stem_l[:, 256 * kx + 128 : 256 * kx + 256] = lo[:, 128 * kx : 128 * kx + 128]
    c["stem_l"] = stem_l

    # w1e[32k + c, 9co + t] = W1[k, co, c, t]*s1[co]*WS
    w1 = np.asarray(i["c1_w"], np.float32) * s1[None, :, None, None, None] * WS
    w1e = w1.transpose(0, 2, 1, 3, 4).reshape(4, 32, 64 * 9).reshape(128, 576)
    c["w1e"] = np.ascontiguousarray(w1e).astype(NP_BF16)

    # w2e_tau[64k' + c, 9co + t] = W2[2tau + k', co, c, t]*s2[co]*WS
    w2 = np.asarray(i["c2_w"], np.float32) * s2[None, :, None, None, None] * WS
    w2p = w2.transpose(0, 2, 1, 3, 4).reshape(4, 64, 128 * 9)
    c["w2ea"] = np.ascontiguousarray(w2p[0:2].reshape(128, 1152)).astype(NP_BF16)
    c["w2eb"] = np.ascontiguousarray(w2p[2:4].reshape(128, 1152)).astype(NP_BF16)

    c["bid1"] = np.tile(np.eye(32, dtype=np.float32), (4, 1)).astype(NP_BF16)
    c["bid2"] = np.tile(np.eye(64, dtype=np.float32), (2, 1)).astype(NP_BF16)
    c["e1x"] = np.repeat(np.eye(4, dtype=np.float32), 32, axis=1)
    c["e2a"] = np.repeat(np.eye(4, dtype=np.float32)[:, 0:2], 64, axis=1)
    c["e2b"] = np.repeat(np.eye(4, dtype=np.float32)[:, 2:4], 64, axis=1)

    c["r1l"] = np.ascontiguousarray((np.asarray(i["r1_w"], np.float32) / 12544.0).T)
    c["r1b"] = np.asarray(i["r1_b"], np.float32).reshape(4, 1)
    c["r2l"] = np.ascontiguousarray((np.asarray(i["r2_w"], np.float32) / 12544.0).T)
    c["r2b"] = np.asarray(i["r2_b"], np.float32).reshape(4, 1)
    c["c1bT"] = np.asarray(i["c1_b"], np.float32) * s1[None, :]
    c["bn1h64"] = h1v.reshape(64, 1)
    c["c2bT"] = np.asarray(i["c2_b"], np.float32) * s2[None, :]
    c["bn2h"] = h2v.reshape(128, 1)
    c["bn0h"] = np.tile(h0v, 4).reshape(128, 1)
    c["fcl"] = np.ascontiguousarray((np.asarray(i["fc_w"], np.float32) / 3136.0).T)
    c["fcb"] = np.asarray(i["fc_b"], np.float32).reshape(2, 1)
    return c


_PROG = None


def kernel(**inputs):
    global _PROG
    if _PROG is None:
        _PROG = build_program()
    nc = _PROG
    c = prep_consts(inputs)
    x = np.asarray(inputs["x"], np.float32).astype(NP_FP8)
    in_maps = []
    for core in range(8):
        m = dict(c)
        m["xs"] = np.ascontiguousarray(x[core * 8 : core * 8 + 8])
        in_maps.append(m)
    res = run_bass_kernel_spmd(nc, in_maps, core_ids=list(range(8)))
    out = np.concatenate([r["out"].T for r in res.results], axis=0)
    return out.astype(np.float32)


# revision 19
# speedup vs baseline: 1.2187x; 1.2187x over previous
import sys

sys.path.insert(0, "/opt/trn_rl_repo")
from contextlib import ExitStack

import numpy as np

import concourse.bass as bass  # noqa: F401
import concourse.mybir as mybir
import concourse.tile as tile
from concourse import bacc
from concourse.bass_utils import run_bass_kernel_spmd

F32 = mybir.dt.float32
F32R = mybir.dt.float32r
BF16 = mybir.dt.bfloat16
FP8 = mybir.dt.float8e4
AF = mybir.ActivationFunctionType
ALU = mybir.AluOpType
AX = mybir.AxisListType
EPS = 1e-5

NP_BF16 = mybir.dt.np(BF16)
NP_FP8 = mybir.dt.np(FP8)


def fr(ap):
    return ap.bitcast(F32R)


def build_program():
    nc = bacc.Bacc(trn_type="TRN2")

    def din(name, shape, dt_=F32):
        return nc.dram_tensor(name, shape, dt_, kind="ExternalInput")

    xrg0_d = din("xrg0", [108, 112 * 228], FP8)
    xrg1_d = din("xrg1", [108, 112 * 228], FP8)
    stem_l = din("stem_l", [108, 128], BF16)
    w1e_d = din("w1e", [128, 576], BF16)
    w2ea_d = din("w2ea", [128, 1152], BF16)
    w2eb_d = din("w2eb", [128, 1152], BF16)
    bid1_d = din("bid1", [128, 32], BF16)
    bid2_d = din("bid2", [128, 64], BF16)
    e1x_d = din("e1x", [4, 128], F32R)
    e2a_d = din("e2a", [4, 128], F32R)
    e2b_d = din("e2b", [4, 128], F32R)
    r1l_d = din("r1l", [32, 4], F32R)
    r1b_d = din("r1b", [4, 1])
    r2l_d = din("r2l", [64, 4], F32R)
    r2b_d = din("r2b", [4, 1])
    c1bT_d = din("c1bT", [4, 64], F32R)
    bn1h_d = din("bn1h64", [64, 1])
    c2bT_d = din("c2bT", [4, 128], F32R)
    bn2h_d = din("bn2h", [128, 1])
    bn0h_d = din("bn0h", [128, 1])
    fcl_d = din("fcl", [128, 2], F32R)
    fcb_d = din("fcb", [2, 1])
    out_d = nc.dram_tensor("out", [2, 8], F32, kind="ExternalOutput")

    with tile.TileContext(nc) as tc, ExitStack() as ctx:
        def P(name, bufs, space="SBUF"):
            return ctx.enter_context(tc.tile_pool(name=name, bufs=bufs, space=space))

        consts = P("consts", 1)
        xrp = P("xr", 1)
        h0p = P("h0", 1)
        h1p = P("h1", 2)
        cw1p = P("cw1", 2)
        cw2p = P("cw2", 2)
        sc1p = P("sc1", 2)
        sc2p = P("sc2", 2)
        scrp = P("scr", 2)
        smallp = P("small", 4)
        gapp = P("gap", 2)
        ps = P("ps", 4, "PSUM")

        def lc(dt_, shape, tag, tdt=F32):
            t = consts.tile(shape, tdt, tag=tag)
            # separate queue from the strided bf16 xs loads: interleaving
            # small f32 DMAs with them on one DGE queue corrupts transfers
            nc.scalar.dma_start(t[:], dt_[:, :])
            return t

        stem_sb = lc(stem_l, [108, 128], "stem_l", BF16)
        w1e = lc(w1e_d, [128, 576], "w1e", BF16)
        w2ea = lc(w2ea_d, [128, 1152], "w2ea", BF16)
        w2eb = lc(w2eb_d, [128, 1152], "w2eb", BF16)
        bid1 = lc(bid1_d, [128, 32], "bid1", BF16)
        bid2 = lc(bid2_d, [128, 64], "bid2", BF16)
        e1x = lc(e1x_d, [4, 128], "e1x", F32R)
        e2a = lc(e2a_d, [4, 128], "e2a", F32R)
        e2b = lc(e2b_d, [4, 128], "e2b", F32R)
        r1l = lc(r1l_d, [32, 4], "r1l", F32R)
        r1b = lc(r1b_d, [4, 1], "r1b")
        r2l = lc(r2l_d, [64, 4], "r2l", F32R)
        r2b = lc(r2b_d, [4, 1], "r2b")
        c1bT = lc(c1bT_d, [4, 64], "c1bT", F32R)
        bn1h = lc(bn1h_d, [64, 1], "bn1h")
        c2bT = lc(c2bT_d, [4, 128], "c2bT", F32R)
        bn2h = lc(bn2h_d, [128, 1], "bn2h")
        bn0h = lc(bn0h_d, [128, 1], "bn0h")
        fcl = lc(fcl_d, [128, 2], "fcl", F32R)
        fcb = lc(fcb_d, [2, 1], "fcb")

        # ---- x pre-packed on host: xr_g[36kx + 9s + 3ky + c, R, C]
        #      = x[4g+s, c, 2R-1+ky, C-2+kx] (zero padded)
        xrA = xrp.tile([108, 112 * 228], FP8, tag="xrA")
        xrB = xrp.tile([108, 112 * 228], FP8, tag="xrB")
        nc.sync.dma_start(xrA[:, :], xrg0_d[:, :])
        nc.sync.dma_start(xrB[:, :], xrg1_d[:, :])
        xrgs = [xrA[:].rearrange("p (r c) -> p r c", c=228),
                xrB[:].rearrange("p (r c) -> p r c", c=228)]
        h0 = h0p.tile([128, 114 * 114], FP8, tag="h0")
        h03 = h0[:].rearrange("p (r c) -> p r c", c=114)
        nc.gpsimd.memset(h03[:, 0:1, :], 0.0)
        nc.gpsimd.memset(h03[:, 113:114, :], 0.0)
        nc.gpsimd.memset(h03[:, :, 0:1], 0.0)
        nc.gpsimd.memset(h03[:, :, 113:114], 0.0)

        pooled1 = smallp.tile([32, 8], F32R, tag="pooled1")
        pooled2 = smallp.tile([64, 8], F32R, tag="pooled2")
        pooled3 = smallp.tile([128, 8], F32R, tag="pooled3")

        for g in range(2):
            # ---------------- stem ----------------
            gap0 = gapp.tile([128, 14], F32, tag="gap0")
            xr3 = xrgs[g]
            for sy in range(14):
                pst = ps.tile([128, 1024], F32, tag="pb")
                for c2 in range(2):
                    y0 = 8 * sy + 4 * c2
                    nc.tensor.matmul(
                        pst[:, 512 * c2 : 512 * c2 + 448],
                        stem_sb[0:108, :],
                        xr3[0:108, y0 : y0 + 4, 1:224:2],
                        start=True,
                        stop=True,
                    )
                nc.scalar.activation(
                    h03[:, 8 * sy + 1 : 8 * sy + 9, 1:113].rearrange(
                        "p (a b) c -> p a b c", a=2
                    ),
                    pst[:, 0:1024].rearrange("p (a b) -> p a b", a=2)[:, :, 0:448]
                    .rearrange("p a (b c) -> p a b c", b=4),
                    AF.Relu,
                    bias=bn0h[:, 0:1],
                    scale=1.0,
                    accum_out=gap0[:, sy : sy + 1],
                )
            g1 = smallp.tile([128, 1], F32, tag="g1")
            nc.vector.tensor_reduce(g1[:], gap0[:], AX.X, ALU.add)
            for s in range(4):
                nc.sync.dma_start(
                    pooled1[:, 4 * g + s : 4 * g + s + 1],
                    g1[32 * s : 32 * s + 32, :].bitcast(F32R),
                )
            # ---------------- routing 1 ----------------
            psr = ps.tile([128, 1024], F32, tag="pb")
            nc.tensor.matmul(
                psr[0:4, 0:4], fr(r1l[:]), fr(pooled1[:, 4 * g : 4 * g + 4]),
                start=True, stop=True,
            )
            r1g = smallp.tile([4, 4], F32R, tag="r1g")
            nc.scalar.activation(r1g[:], psr[0:4, 0:4], AF.Sigmoid,
                                 bias=r1b[:, 0:1], scale=1.0)
            # broadcast r over (k, c) partitions: r1full[32k+c, s] = r1g[k, s]
            psb = ps.tile([128, 1024], F32, tag="pb")
            nc.tensor.matmul(psb[:, 0:4], fr(e1x[:]), fr(r1g[:]), start=True, stop=True)
            r1full = smallp.tile([128, 4], F32, tag="r1full")
            nc.scalar.copy(r1full[:], psb[:, 0:4])
            # bias1[co, s] = (r . c1b*s1)[co, s] + bn1h[co]
            psc = ps.tile([128, 1024], F32, tag="pb")
            nc.tensor.matmul(psc[0:64, 0:4], fr(c1bT[:]), fr(r1g[:]), start=True, stop=True)
            bias1 = smallp.tile([64, 4], F32, tag="bias1")
            nc.scalar.activation(bias1[:], psc[0:64, 0:4], AF.Identity,
                                 bias=bn1h[:, 0:1], scale=1.0)
            # ---------------- combine conv1 weights ----------------
            # cwt[64pr + 32sl + c, (64sl+co)*9 + t] = sum_k r[j,k] W1[k,co,c,t]
            cwt = cw1p.tile([128, 1152], BF16, tag="cwt")
            for pr in range(2):
                for sl in range(2):
                    nc.gpsimd.memset(
                        cwt[64 * pr + 32 * sl : 64 * pr + 32 * sl + 32,
                            (1 - sl) * 576 : (2 - sl) * 576], 0.0)
            psw = ps.tile([128, 1024], F32, tag="pb")
            for j in range(4):
                tj = sc1p.tile([128, 576], BF16, tag="sc1")
                nc.vector.tensor_scalar(tj[:], w1e[:], r1full[:, j : j + 1], None, ALU.mult)
                for h in range(2):
                    nc.tensor.matmul(
                        psw[32 * j : 32 * j + 32, 512 * h : 512 * h + 288],
                        bid1[:], tj[:, 288 * h : 288 * h + 288],
                        start=True, stop=True, tile_position=(0, 32 * j),
                    )
            for pr in range(2):
                for sl in range(2):
                    j = 2 * pr + sl
                    nc.scalar.copy(
                        cwt[64 * pr + 32 * sl : 64 * pr + 32 * sl + 32,
                            576 * sl : 576 * sl + 576].rearrange(
                                "p (a b) -> p a b", a=2),
                        psw[32 * j : 32 * j + 32, 0:1024].rearrange(
                            "p (a b) -> p a b", a=2)[:, :, 0:288],
                    )
            for pr in range(2):
                # ---------------- conv1 for sample pair ----------------
                pbias = smallp.tile([128, 1], F32, tag="pbias")
                nc.sync.dma_start(pbias[0:64, :], bias1[:, 2 * pr : 2 * pr + 1])
                nc.sync.dma_start(pbias[64:128, :], bias1[:, 2 * pr + 1 : 2 * pr + 2])
                h1 = h1p.tile([128, 114 * 114], FP8, tag="h1")
                h13 = h1[:].rearrange("p (r c) -> p r c", c=114)
                nc.gpsimd.memset(h13[:, 0:1, :], 0.0)
                nc.gpsimd.memset(h13[:, 113:114, :], 0.0)
                nc.gpsimd.memset(h13[:, :, 0:1], 0.0)
                nc.gpsimd.memset(h13[:, :, 113:114], 0.0)
                gap1 = gapp.tile([128, 14], F32, tag="gap1")
                for grp in range(14):
                    pst = ps.tile([128, 1024], F32, tag="pb")
                    for c2 in range(2):
                        y0 = 8 * grp + 4 * c2
                        for t in range(9):
                            ky, kx = t // 3, t % 3
                            nc.tensor.matmul(
                                pst[:, 512 * c2 : 512 * c2 + 448],
                                cwt[64 * pr : 64 * pr + 64, t : 1152 : 9],
                                h03[64 * pr : 64 * pr + 64,
                                    y0 + ky : y0 + ky + 4, kx : kx + 112],
                                start=(t == 0),
                                stop=(t == 8),
                            )
                    nc.scalar.activation(
                        h13[:, 8 * grp + 1 : 8 * grp + 9, 1:113].rearrange(
                            "p (a b) c -> p a b c", a=2),
                        pst[:, 0:1024].rearrange("p (a b) -> p a b", a=2)[:, :, 0:448]
                        .rearrange("p a (b c) -> p a b c", b=4),
                        AF.Relu,
                        bias=pbias[:, 0:1],
                        scale=1.0,
                        accum_out=gap1[:, grp : grp + 1],
                    )
                g2 = smallp.tile([128, 1], F32, tag="g2")
                nc.vector.tensor_reduce(g2[:], gap1[:], AX.X, ALU.add)
                col0 = 4 * g + 2 * pr
                nc.sync.dma_start(pooled2[:, col0 : col0 + 1], g2[0:64, :].bitcast(F32R))
                nc.sync.dma_start(pooled2[:, col0 + 1 : col0 + 2], g2[64:128, :].bitcast(F32R))
                # ---------------- routing 2 ----------------
                ps2 = ps.tile([128, 1024], F32, tag="pb")
                nc.tensor.matmul(ps2[0:4, 0:2], fr(r2l[:]),
                                 fr(pooled2[:, col0 : col0 + 2]), start=True, stop=True)
                r2g = smallp.tile([4, 2], F32R, tag="r2g")
                nc.scalar.activation(r2g[:], ps2[0:4, 0:2], AF.Sigmoid,
                                     bias=r2b[:, 0:1], scale=1.0)
                # r2full[64j+c, 2*tau+s] = r2g[2*tau+j, s]
                psb2 = ps.tile([128, 1024], F32, tag="pb")
                nc.tensor.matmul(psb2[:, 0:2], fr(e2a[:]), fr(r2g[:]), start=True, stop=True)
                nc.tensor.matmul(psb2[:, 512:514], fr(e2b[:]), fr(r2g[:]), start=True, stop=True)
                r2full = smallp.tile([128, 4], F32, tag="r2full")
                nc.scalar.copy(
                    r2full[:].rearrange("p (a b) -> p a b", a=2),
                    psb2[:, 0:1024].rearrange("p (a b) -> p a b", a=2)[:, :, 0:2],
                )
                # bias2[co', s] with bn2h
                psc2 = ps.tile([128, 1024], F32, tag="pb")
                nc.tensor.matmul(psc2[:, 0:2], fr(c2bT[:]), fr(r2g[:]), start=True, stop=True)
                bias2 = smallp.tile([128, 2], F32, tag="bias2")
                nc.scalar.activation(bias2[:], psc2[:, 0:2], AF.Identity,
                                     bias=bn2h[:, 0:1], scale=1.0)
                for sl in range(2):
                    # ---------------- combine conv2 weights ----------------
                    cw2f = cw2p.tile([128, 1152], BF16, tag="cw2f")
                    ta = sc2p.tile([128, 1152], BF16, tag="sc2")
                    nc.vector.tensor_scalar(ta[:], w2ea[:], r2full[:, sl : sl + 1],
                                            None, ALU.mult)
                    tb = sc2p.tile([128, 1152], BF16, tag="sc2")
                    nc.vector.tensor_scalar(tb[:], w2eb[:], r2full[:, 2 + sl : 3 + sl],
                                            None, ALU.mult)
                    for half in range(2):
                        psw2 = ps.tile([128, 1024], F32, tag="pb")
                        for q in range(2):
                            qq = 2 * half + q
                            nc.tensor.matmul(
                                psw2[64 * sl : 64 * sl + 64, 512 * q : 512 * q + 288],
                                bid2[:], ta[:, 288 * qq : 288 * qq + 288],
                                start=True, stop=False, tile_position=(0, 64 * sl),
                            )
                            nc.tensor.matmul(
                                psw2[64 * sl : 64 * sl + 64, 512 * q : 512 * q + 288],
                                bid2[:], tb[:, 288 * qq : 288 * qq + 288],
                                start=False, stop=True, tile_position=(0, 64 * sl),
                            )
                        nc.scalar.copy(
                            cw2f[64 * sl : 64 * sl + 64,
                                 576 * half : 576 * half + 576].rearrange(
                                     "p (a b) -> p a b", a=2),
                            psw2[64 * sl : 64 * sl + 64, 0:1024].rearrange(
                                "p (a b) -> p a b", a=2)[:, :, 0:288],
                        )
                    # ---------------- conv2 ----------------
                    gap2 = gapp.tile([128, 4], F32, tag="gap2")
                    for grp in range(4):
                        nch = 2 if grp < 3 else 1
                        pst = ps.tile([128, 1024], F32, tag="pb")
                        for c2 in range(nch):
                            y0 = 16 * (2 * grp + c2)
                            for t in range(9):
                                ky, kx = t // 3, t % 3
                                nc.tensor.matmul(
                                    pst[:, 512 * c2 : 512 * c2 + 448],
                                    cw2f[64 * sl : 64 * sl + 64, t : 1152 : 9],
                                    h13[64 * sl : 64 * sl + 64,
                                        y0 + ky : y0 + ky + 16 : 2,
                                        kx : kx + 112 : 2],
                                    start=(t == 0),
                                    stop=(t == 8),
                                )
                        scr = scrp.tile([128, 896], BF16, tag="scr")
                        if nch == 2:
                            nc.scalar.activation(
                                scr[:].rearrange("p (a b) -> p a b", a=2),
                                pst[:, 0:1024].rearrange("p (a b) -> p a b", a=2)
                                [:, :, 0:448],
                                AF.Relu,
                                bias=bias2[:, sl : sl + 1],
                                scale=1.0,
                                accum_out=gap2[:, grp : grp + 1],
                            )
                        else:
                            nc.scalar.activation(
                                scr[:, 0:448],
                                pst[:, 0:448],
                                AF.Relu,
                                bias=bias2[:, sl : sl + 1],
                                scale=1.0,
                                accum_out=gap2[:, grp : grp + 1],
                            )
                    g3 = smallp.tile([128, 1], F32, tag="g3")
                    nc.vector.tensor_reduce(g3[:], gap2[:], AX.X, ALU.add)
                    scol = 4 * g + 2 * pr + sl
                    nc.sync.dma_start(pooled3[:, scol : scol + 1], g3[:].bitcast(F32R))
        # ---------------- head ----------------
        psf = ps.tile([128, 1024], F32, tag="pb")
        nc.tensor.matmul(psf[0:2, 0:8], fr(fcl[:]), fr(pooled3[:]), start=True, stop=True)
        outsb = smallp.tile([2, 8], F32, tag="outsb")
        nc.scalar.activation(outsb[:], psf[0:2, 0:8], AF.Identity,
                             bias=fcb[:, 0:1], scale=1.0)
        nc.sync.dma_start(out_d[:, :], outsb[:])
    nc.finalize()
    return nc


def prep_consts(i):
    def bn(g, b, m, v):
        sc = g / np.sqrt(v + EPS)
        return sc.astype(np.float32), (b - m * sc).astype(np.float32)

    c = {}
    s0, h0v = bn(*[np.asarray(i[k], np.float32) for k in ("bn0_g", "bn0_b", "bn0_m", "bn0_v")])
    s1, h1v = bn(*[np.asarray(i[k], np.float32) for k in ("bn1_g", "bn1_b", "bn1_m", "bn1_v")])
    s2, h2v = bn(*[np.asarray(i[k], np.float32) for k in ("bn2_g", "bn2_b", "bn2_m", "bn2_v")])

    # stem_l[36kx + 9s + 3ky + c, 32s + co] = W[co,c,ky,kx]*s0[co]
    sw = np.asarray(i["stem_w"], np.float32) * s0[:, None, None, None]  # [32,3,3,3]
    base = sw.transpose(2, 1, 3, 0)  # [ky, c, kx, co]
    stem_l = np.zeros((108, 128), np.float32)
    for kx in range(3):
        for s in range(4):
            for ky in range(3):
                for cc in range(3):
                    stem_l[36 * kx + 9 * s + 3 * ky + cc,
                           32 * s : 32 * s + 32] = base[ky, cc, kx]
    c["stem_l"] = stem_l.astype(NP_BF16)

    # w1e[32k + c, 9co + 3ky + kx] = W1[k, co, c, ky, kx]*s1[co]
    w1 = np.asarray(i["c1_w"], np.float32) * s1[None, :, None, None, None]  # [4,64,32,3,3]
    w1e = w1.transpose(0, 2, 1, 3, 4).reshape(4, 32, 64 * 9).reshape(128, 576)
    c["w1e"] = np.ascontiguousarray(w1e).astype(NP_BF16)

    # w2e_tau[64k' + c, 9co + t] = W2[2tau + k', co, c, t]*s2[co]
    w2 = np.asarray(i["c2_w"], np.float32) * s2[None, :, None, None, None]  # [4,128,64,3,3]
    w2p = w2.transpose(0, 2, 1, 3, 4).reshape(4, 64, 128 * 9)
    c["w2ea"] = np.ascontiguousarray(w2p[0:2].reshape(128, 1152)).astype(NP_BF16)
    c["w2eb"] = np.ascontiguousarray(w2p[2:4].reshape(128, 1152)).astype(NP_BF16)

    c["bid1"] = np.tile(np.eye(32, dtype=np.float32), (4, 1)).astype(NP_BF16)
    c["bid2"] = np.tile(np.eye(64, dtype=np.float32), (2, 1)).astype(NP_BF16)
    c["e1x"] = np.repeat(np.eye(4, dtype=np.float32), 32, axis=1)
    c["e2a"] = np.repeat(np.eye(4, dtype=np.float32)[:, 0:2], 64, axis=1)
    c["e2b"] = np.repeat(np.eye(4, dtype=np.float32)[:, 2:4], 64, axis=1)

    c["r1l"] = np.ascontiguousarray((np.asarray(i["r1_w"], np.float32) / 12544.0).T)
    c["r1b"] = np.asarray(i["r1_b"], np.float32).reshape(4, 1)
    c["r2l"] = np.ascontiguousarray((np.asarray(i["r2_w"], np.float32) / 12544.0).T)
    c["r2b"] = np.asarray(i["r2_b"], np.float32).reshape(4, 1)
    c["c1bT"] = np.asarray(i["c1_b"], np.float32) * s1[None, :]
    c["bn1h64"] = h1v.reshape(64, 1)
    c["c2bT"] = np.asarray(i["c2_b"], np.float32) * s2[None, :]
    c["bn2h"] = h2v.reshape(128, 1)
    c["bn0h"] = np.tile(h0v, 4).reshape(128, 1)
    c["fcl"] = np.ascontiguousarray((np.asarray(i["fc_w"], np.float32) / 3136.0).T)
    c["fcb"] = np.asarray(i["fc_b"], np.float32).reshape(2, 1)
    return c


_PROG = None


def pack_xr(x8):
    # x8: [8, 3, 224, 224] float32 (one core's samples)
    out = np.zeros((2, 108, 112, 228), np.float32)
    for g in range(2):
        for kx in range(3):
            cl, ch = max(0, 2 - kx), min(228, 226 - kx + 2)
            # valid C range: [2-kx, 226-kx)
            c0 = 2 - kx if kx < 2 else 0
            c1 = 226 - kx
            xl0 = c0 - 2 + kx  # first x col
            for s in range(4):
                xi = x8[4 * g + s]
                b = 36 * kx + 9 * s
                out[g, b + 0 : b + 3, 1:112, c0:c1] = xi[:, 1:222:2, xl0 : xl0 + (c1 - c0)]
                out[g, b + 3 : b + 6, 0:112, c0:c1] = xi[:, 0:223:2, xl0 : xl0 + (c1 - c0)]
                out[g, b + 6 : b + 9, 0:112, c0:c1] = xi[:, 1:224:2, xl0 : xl0 + (c1 - c0)]
    return out.reshape(2, 108, 112 * 228).astype(NP_FP8)


def kernel(**inputs):
    global _PROG
    if _PROG is None:
        _PROG = build_program()
    nc = _PROG
    c = prep_consts(inputs)
    x = np.asarray(inputs["x"], np.float32)
    in_maps = []
    for core in range(8):
        m = dict(c)
        xp = pack_xr(x[core * 8 : core * 8 + 8])
        m["xrg0"] = np.ascontiguousarray(xp[0])
        m["xrg1"] = np.ascontiguousarray(xp[1])
        in_maps.append(m)
    res = run_bass_kernel_spmd(nc, in_maps, core_ids=list(range(8)))
    out = np.concatenate([r["out"].T for r in res.results], axis=0)
    return out.astype(np.float32)


# revision 20
# speedup vs baseline: 1.9891x; 1.6322x over previous
import sys

sys.path.insert(0, "/opt/trn_rl_repo")
from contextlib import ExitStack

import numpy as np

import concourse.bass as bass  # noqa: F401
import concourse.mybir as mybir
import concourse.tile as tile
from concourse import bacc
from concourse.ap import AP
from concourse.bass_utils import run_bass_kernel_spmd

F32 = mybir.dt.float32
F32R = mybir.dt.float32r
BF16 = mybir.dt.bfloat16
FP8 = mybir.dt.float8e4
AF = mybir.ActivationFunctionType
ALU = mybir.AluOpType
AX = mybir.AxisListType
EPS = 1e-5

NP_BF16 = mybir.dt.np(BF16)
NP_FP8 = mybir.dt.np(FP8)


def fr(ap):
    return ap.bitcast(F32R)


def mk(t, fs, base, n, off, dims):
    """Manual AP on tile t: partitions [base, base+n), free offset off, dims."""
    return AP(t[:].tensor, base * fs + off, [[fs, n]] + [list(d) for d in dims])


def build_program():
    nc = bacc.Bacc(trn_type="TRN2")

    def din(name, shape, dt_=F32):
        return nc.dram_tensor(name, shape, dt_, kind="ExternalInput")

    xrg0_d = din("xrg0", [108, 112 * 228], FP8)
    xrg1_d = din("xrg1", [108, 112 * 228], FP8)
    stem_l = din("stem_l", [108, 128], BF16)
    w1e_d = din("w1e", [128, 576], BF16)
    w2ea_d = din("w2ea", [128, 1152], BF16)
    w2eb_d = din("w2eb", [128, 1152], BF16)
    bid1_d = din("bid1", [128, 96], BF16)
    bid2_d = din("bid2", [128, 64], BF16)
    e1x_d = din("e1x", [4, 128], F32R)
    e2a_d = din("e2a", [4, 128], F32R)
    e2b_d = din("e2b", [4, 128], F32R)
    r1l_d = din("r1l", [32, 4], F32R)
    r1b_d = din("r1b", [4, 1])
    r2l_d = din("r2l", [64, 4], F32R)
    r2b_d = din("r2b", [4, 1])
    c1bT_d = din("c1bT", [4, 64], F32R)
    bn1h_d = din("bn1h64", [64, 1])
    c2bT_d = din("c2bT", [4, 128], F32R)
    bn2h_d = din("bn2h", [128, 1])
    bn0h_d = din("bn0h", [128, 1])
    fcl_d = din("fcl", [128, 2], F32R)
    fcb_d = din("fcb", [2, 1])
    out_d = nc.dram_tensor("out", [2, 8], F32, kind="ExternalOutput")

    with tile.TileContext(nc) as tc, ExitStack() as ctx:
        def P(name, bufs, space="SBUF"):
            return ctx.enter_context(tc.tile_pool(name=name, bufs=bufs, space=space))

        consts = P("consts", 1)
        xrp = P("xr", 1)
        h0p = P("h0", 1)
        h1p = P("h1", 2)
        cw1p = P("cw1", 2)
        h0sp = P("h0s", 4)
        cw2p = P("cw2", 2)
        sc1p = P("sc1", 2)
        sc2p = P("sc2", 2)
        scrp = P("scr", 2)
        smallp = P("small", 4)
        gapp = P("gap", 2)
        ps = P("ps", 4, "PSUM")

        def lc(dt_, shape, tag, tdt=F32):
            t = consts.tile(shape, tdt, tag=tag)
            # separate queue from the strided bf16 xs loads: interleaving
            # small f32 DMAs with them on one DGE queue corrupts transfers
            nc.scalar.dma_start(t[:], dt_[:, :])
            return t

        stem_sb = lc(stem_l, [108, 128], "stem_l", BF16)
        w1e = lc(w1e_d, [128, 576], "w1e", BF16)
        w2ea = lc(w2ea_d, [128, 1152], "w2ea", BF16)
        w2eb = lc(w2eb_d, [128, 1152], "w2eb", BF16)
        bid1 = lc(bid1_d, [128, 96], "bid1", BF16)
        bid2 = lc(bid2_d, [128, 64], "bid2", BF16)
        e1x = lc(e1x_d, [4, 128], "e1x", F32R)
        e2a = lc(e2a_d, [4, 128], "e2a", F32R)
        e2b = lc(e2b_d, [4, 128], "e2b", F32R)
        r1l = lc(r1l_d, [32, 4], "r1l", F32R)
        r1b = lc(r1b_d, [4, 1], "r1b")
        r2l = lc(r2l_d, [64, 4], "r2l", F32R)
        r2b = lc(r2b_d, [4, 1], "r2b")
        c1bT = lc(c1bT_d, [4, 64], "c1bT", F32R)
        bn1h = lc(bn1h_d, [64, 1], "bn1h")
        c2bT = lc(c2bT_d, [4, 128], "c2bT", F32R)
        bn2h = lc(bn2h_d, [128, 1], "bn2h")
        bn0h = lc(bn0h_d, [128, 1], "bn0h")
        fcl = lc(fcl_d, [128, 2], "fcl", F32R)
        fcb = lc(fcb_d, [2, 1], "fcb")

        # ---- x pre-packed on host: xr_g[36kx + 9s + 3ky + c, R, C]
        #      = x[4g+s, c, 2R-1+ky, C-2+kx] (zero padded)
        xrA = xrp.tile([108, 112 * 228], FP8, tag="xrA")
        xrB = xrp.tile([108, 112 * 228], FP8, tag="xrB")
        nc.sync.dma_start(xrA[:, :], xrg0_d[:, :])
        nc.sync.dma_start(xrB[:, :], xrg1_d[:, :])
        xrgs = [xrA[:].rearrange("p (r c) -> p r c", c=228),
                xrB[:].rearrange("p (r c) -> p r c", c=228)]
        h0 = h0p.tile([128, 114 * 114], FP8, tag="h0")
        h03 = h0[:].rearrange("p (r c) -> p r c", c=114)
        nc.gpsimd.memset(h03[:, 0:1, :], 0.0)
        nc.gpsimd.memset(h03[:, 113:114, :], 0.0)
        nc.gpsimd.memset(h03[:, :, 0:1], 0.0)
        nc.gpsimd.memset(h03[:, :, 113:114], 0.0)

        pooled1 = smallp.tile([32, 8], F32R, tag="pooled1")
        pooled2 = smallp.tile([64, 8], F32R, tag="pooled2")
        pooled3 = smallp.tile([128, 8], F32R, tag="pooled3")

        for g in range(2):
            # ---------------- stem ----------------
            gap0 = gapp.tile([128, 14], F32, tag="gap0")
            xr3 = xrgs[g]
            for sy in range(14):
                pst = ps.tile([128, 1024], F32, tag="pb")
                for c2 in range(2):
                    y0 = 8 * sy + 4 * c2
                    nc.tensor.matmul(
                        pst[:, 512 * c2 : 512 * c2 + 448],
                        stem_sb[0:108, :],
                        xr3[0:108, y0 : y0 + 4, 1:224:2],
                        start=True,
                        stop=True,
                    )
                nc.scalar.activation(
                    h03[:, 8 * sy + 1 : 8 * sy + 9, 1:113].rearrange(
                        "p (a b) c -> p a b c", a=2
                    ),
                    pst[:, 0:1024].rearrange("p (a b) -> p a b", a=2)[:, :, 0:448]
                    .rearrange("p a (b c) -> p a b c", b=4),
                    AF.Relu,
                    bias=bn0h[:, 0:1],
                    scale=1.0,
                    accum_out=gap0[:, sy : sy + 1],
                )
            g1 = smallp.tile([128, 1], F32, tag="g1")
            nc.vector.tensor_reduce(g1[:], gap0[:], AX.X, ALU.add)
            for s in range(4):
                nc.sync.dma_start(
                    pooled1[:, 4 * g + s : 4 * g + s + 1],
                    g1[32 * s : 32 * s + 32, :].bitcast(F32R),
                )
            # ---------------- routing 1 ----------------
            psr = ps.tile([128, 1024], F32, tag="pb")
            nc.tensor.matmul(
                psr[0:4, 0:4], fr(r1l[:]), fr(pooled1[:, 4 * g : 4 * g + 4]),
                start=True, stop=True,
            )
            r1g = smallp.tile([4, 4], F32R, tag="r1g")
            nc.scalar.activation(r1g[:], psr[0:4, 0:4], AF.Sigmoid,
                                 bias=r1b[:, 0:1], scale=1.0)
            # broadcast r over (k, c) partitions: r1full[32k+c, s] = r1g[k, s]
            psb = ps.tile([128, 1024], F32, tag="pb")
            nc.tensor.matmul(psb[:, 0:4], fr(e1x[:]), fr(r1g[:]), start=True, stop=True)
            r1full = smallp.tile([128, 4], F32, tag="r1full")
            nc.scalar.copy(r1full[:], psb[:, 0:4])
            # bias1[co, s] = (r . c1b*s1)[co, s] + bn1h[co]
            psc = ps.tile([128, 1024], F32, tag="pb")
            nc.tensor.matmul(psc[0:64, 0:4], fr(c1bT[:]), fr(r1g[:]), start=True, stop=True)
            bias1 = smallp.tile([64, 4], F32, tag="bias1")
            nc.scalar.activation(bias1[:], psc[0:64, 0:4], AF.Identity,
                                 bias=bn1h[:, 0:1], scale=1.0)
            # -------- combine conv1 weights, ky-packed per sample --------
            # cwt_s[32ky + c, 64kx + co] = sum_k r[j,k] W1[k,co,c,ky,kx]
            cwts_g = []
            for j in range(4):
                tj = sc1p.tile([128, 576], BF16, tag="sc1")
                nc.vector.tensor_scalar(tj[:], w1e[:], r1full[:, j : j + 1], None, ALU.mult)
                psw = ps.tile([128, 1024], F32, tag="pb")
                for h in range(2):
                    nc.tensor.matmul(
                        psw[0:96, 512 * h : 512 * h + 288],
                        bid1[:], tj[:, 288 * h : 288 * h + 288],
                        start=True, stop=True, tile_position=(0, 0),
                    )
                cws = cw1p.tile([96, 192], BF16, tag="cwt", bufs=8)
                for b in range(3):
                    nc.scalar.copy(
                        mk(cws, 192, 32 * b, 32, 0, [(32, 2), (1, 32), (64, 3)]),
                        AP(psw.tensor, (32 * b) * 1024 + 3 * b,
                           [[1024, 32], [512, 2], [9, 32], [1, 3]]),
                    )
                cwts_g.append(cws)
            for pr in range(2):
                # ---------------- conv1 for sample pair ----------------
                pbias = smallp.tile([128, 1], F32, tag="pbias")
                nc.sync.dma_start(pbias[0:64, :], bias1[:, 2 * pr : 2 * pr + 1])
                nc.sync.dma_start(pbias[64:128, :], bias1[:, 2 * pr + 1 : 2 * pr + 2])
                h1 = h1p.tile([128, 114 * 114], FP8, tag="h1")
                h13 = h1[:].rearrange("p (r c) -> p r c", c=114)
                nc.gpsimd.memset(h13[:, 0:1, :], 0.0)
                nc.gpsimd.memset(h13[:, 113:114, :], 0.0)
                nc.gpsimd.memset(h13[:, :, 0:1], 0.0)
                nc.gpsimd.memset(h13[:, :, 113:114], 0.0)
                gap1 = gapp.tile([128, 14], F32, tag="gap1")
                h0s3 = []
                for sl in range(2):
                    h0sj = h0sp.tile([96, 114 * 114], FP8, tag="h0s")
                    for b in range(3):
                        nc.sync.dma_start(
                            h0sj[32 * b : 32 * b + 32, 0 : 12996 - 114 * b],
                            h0[64 * pr + 32 * sl : 64 * pr + 32 * sl + 32,
                               114 * b : 12996])
                    h0s3.append(h0sj[:].rearrange("p (r c) -> p r c", c=114))
                for grp in range(14):
                    pst = ps.tile([128, 1024], F32, tag="pb")
                    for c2 in range(2):
                        y0 = 8 * grp + 4 * c2
                        for sl in range(2):
                            cws = cwts_g[2 * pr + sl]
                            for kx in range(3):
                                nc.tensor.matmul(
                                    pst[64 * sl : 64 * sl + 64,
                                        512 * c2 : 512 * c2 + 448],
                                    cws[0:96, 64 * kx : 64 * kx + 64],
                                    h0s3[sl][0:96, y0 : y0 + 4, kx : kx + 112],
                                    start=(kx == 0),
                                    stop=(kx == 2),
                                    tile_position=(0, 64 * sl),
                                )
                    nc.scalar.activation(
                        h13[:, 8 * grp + 1 : 8 * grp + 9, 1:113].rearrange(
                            "p (a b) c -> p a b c", a=2),
                        pst[:, 0:1024].rearrange("p (a b) -> p a b", a=2)[:, :, 0:448]
                        .rearrange("p a (b c) -> p a b c", b=4),
                        AF.Relu,
                        bias=pbias[:, 0:1],
                        scale=1.0,
                        accum_out=gap1[:, grp : grp + 1],
                    )
                g2 = smallp.tile([128, 1], F32, tag="g2")
                nc.vector.tensor_reduce(g2[:], gap1[:], AX.X, ALU.add)
                col0 = 4 * g + 2 * pr
                nc.sync.dma_start(pooled2[:, col0 : col0 + 1], g2[0:64, :].bitcast(F32R))
                nc.sync.dma_start(pooled2[:, col0 + 1 : col0 + 2], g2[64:128, :].bitcast(F32R))
                # ---------------- routing 2 ----------------
                ps2 = ps.tile([128, 1024], F32, tag="pb")
                nc.tensor.matmul(ps2[0:4, 0:2], fr(r2l[:]),
                                 fr(pooled2[:, col0 : col0 + 2]), start=True, stop=True)
                r2g = smallp.tile([4, 2], F32R, tag="r2g")
                nc.scalar.activation(r2g[:], ps2[0:4, 0:2], AF.Sigmoid,
                                     bias=r2b[:, 0:1], scale=1.0)
                # r2full[64j+c, 2*tau+s] = r2g[2*tau+j, s]
                psb2 = ps.tile([128, 1024], F32, tag="pb")
                nc.tensor.matmul(psb2[:, 0:2], fr(e2a[:]), fr(r2g[:]), start=True, stop=True)
                nc.tensor.matmul(psb2[:, 512:514], fr(e2b[:]), fr(r2g[:]), start=True, stop=True)
                r2full = smallp.tile([128, 4], F32, tag="r2full")
                nc.scalar.copy(
                    r2full[:].rearrange("p (a b) -> p a b", a=2),
                    psb2[:, 0:1024].rearrange("p (a b) -> p a b", a=2)[:, :, 0:2],
                )
                # bias2[co', s] with bn2h
                psc2 = ps.tile([128, 1024], F32, tag="pb")
                nc.tensor.matmul(psc2[:, 0:2], fr(c2bT[:]), fr(r2g[:]), start=True, stop=True)
                bias2 = smallp.tile([128, 2], F32, tag="bias2")
                nc.scalar.activation(bias2[:], psc2[:, 0:2], AF.Identity,
                                     bias=bn2h[:, 0:1], scale=1.0)
                for sl in range(2):
                    # ---------------- combine conv2 weights ----------------
                    cw2f = cw2p.tile([128, 1152], BF16, tag="cw2f")
                    ta = sc2p.tile([128, 1152], BF16, tag="sc2")
                    nc.vector.tensor_scalar(ta[:], w2ea[:], r2full[:, sl : sl + 1],
                                            None, ALU.mult)
                    tb = sc2p.tile([128, 1152], BF16, tag="sc2")
                    nc.vector.tensor_scalar(tb[:], w2eb[:], r2full[:, 2 + sl : 3 + sl],
                                            None, ALU.mult)
                    for half in range(2):
                        psw2 = ps.tile([128, 1024], F32, tag="pb")
                        for q in range(2):
                            qq = 2 * half + q
                            nc.tensor.matmul(
                                psw2[64 * sl : 64 * sl + 64, 512 * q : 512 * q + 288],
                                bid2[:], ta[:, 288 * qq : 288 * qq + 288],
                                start=True, stop=False, tile_position=(0, 64 * sl),
                            )
                            nc.tensor.matmul(
                                psw2[64 * sl : 64 * sl + 64, 512 * q : 512 * q + 288],
                                bid2[:], tb[:, 288 * qq : 288 * qq + 288],
                                start=False, stop=True, tile_position=(0, 64 * sl),
                            )
                        nc.scalar.copy(
                            cw2f[64 * sl : 64 * sl + 64,
                                 576 * half : 576 * half + 576].rearrange(
                                     "p (a b) -> p a b", a=2),
                            psw2[64 * sl : 64 * sl + 64, 0:1024].rearrange(
                                "p (a b) -> p a b", a=2)[:, :, 0:288],
                        )
                    # ---------------- conv2 ----------------
                    gap2 = gapp.tile([128, 4], F32, tag="gap2")
                    for grp in range(4):
                        nch = 2 if grp < 3 else 1
                        pst = ps.tile([128, 1024], F32, tag="pb")
                        for c2 in range(nch):
                            y0 = 16 * (2 * grp + c2)
                            for t in range(9):
                                ky, kx = t // 3, t % 3
                                nc.tensor.matmul(
                                    pst[:, 512 * c2 : 512 * c2 + 448],
                                    cw2f[64 * sl : 64 * sl + 64, t : 1152 : 9],
                                    h13[64 * sl : 64 * sl + 64,
                                        y0 + ky : y0 + ky + 16 : 2,
                                        kx : kx + 112 : 2],
                                    start=(t == 0),
                                    stop=(t == 8),
                                )
                        scr = scrp.tile([128, 896], BF16, tag="scr")
                        if nch == 2:
                            nc.scalar.activation(
                                scr[:].rearrange("p (a b) -> p a b", a=2),
                                pst[:, 0:1024].rearrange("p (a b) -> p a b", a=2)
                                [:, :, 0:448],
                                AF.Relu,
                                bias=bias2[:, sl : sl + 1],
                                scale=1.0,
                                accum_out=gap2[:, grp : grp + 1],
                            )
                        else:
                            nc.scalar.activation(
                                scr[:, 0:448],
                                pst[:, 0:448],
                                AF.Relu,
                                bias=bias2[:, sl : sl + 1],
                                scale=1.0,
                                accum_out=gap2[:, grp : grp + 1],
                            )
                    g3 = smallp.tile([128, 1], F32, tag="g3")
                    nc.vector.tensor_reduce(g3[:], gap2[:], AX.X, ALU.add)
                    scol = 4 * g + 2 * pr + sl
                    nc.sync.dma_start(pooled3[:, scol : scol + 1], g3[:].bitcast(F32R))
        # ---------------- head ----------------
        psf = ps.tile([128, 1024], F32, tag="pb")
        nc.tensor.matmul(psf[0:2, 0:8], fr(fcl[:]), fr(pooled3[:]), start=True, stop=True)
        outsb = smallp.tile([2, 8], F32, tag="outsb")
        nc.scalar.activation(outsb[:], psf[0:2, 0:8], AF.Identity,
                             bias=fcb[:, 0:1], scale=1.0)
        nc.sync.dma_start(out_d[:, :], outsb[:])
    nc.finalize()
    return nc


def prep_consts(i):
    def bn(g, b, m, v):
        sc = g / np.sqrt(v + EPS)
        return sc.astype(np.float32), (b - m * sc).astype(np.float32)

    c = {}
    s0, h0v = bn(*[np.asarray(i[k], np.float32) for k in ("bn0_g", "bn0_b", "bn0_m", "bn0_v")])
    s1, h1v = bn(*[np.asarray(i[k], np.float32) for k in ("bn1_g", "bn1_b", "bn1_m", "bn1_v")])
    s2, h2v = bn(*[np.asarray(i[k], np.float32) for k in ("bn2_g", "bn2_b", "bn2_m", "bn2_v")])

    # stem_l[36kx + 9s + 3ky + c, 32s + co] = W[co,c,ky,kx]*s0[co]
    sw = np.asarray(i["stem_w"], np.float32) * s0[:, None, None, None]  # [32,3,3,3]
    base = sw.transpose(2, 1, 3, 0)  # [ky, c, kx, co]
    stem_l = np.zeros((108, 128), np.float32)
    for kx in range(3):
        for s in range(4):
            for ky in range(3):
                for cc in range(3):
                    stem_l[36 * kx + 9 * s + 3 * ky + cc,
                           32 * s : 32 * s + 32] = base[ky, cc, kx]
    c["stem_l"] = stem_l.astype(NP_BF16)

    # w1e[32k + c, 9co + 3ky + kx] = W1[k, co, c, ky, kx]*s1[co]
    w1 = np.asarray(i["c1_w"], np.float32) * s1[None, :, None, None, None]  # [4,64,32,3,3]
    w1e = w1.transpose(0, 2, 1, 3, 4).reshape(4, 32, 64 * 9).reshape(128, 576)
    c["w1e"] = np.ascontiguousarray(w1e).astype(NP_BF16)

    # w2e_tau[64k' + c, 9co + t] = W2[2tau + k', co, c, t]*s2[co]
    w2 = np.asarray(i["c2_w"], np.float32) * s2[None, :, None, None, None]  # [4,128,64,3,3]
    w2p = w2.transpose(0, 2, 1, 3, 4).reshape(4, 64, 128 * 9)
    c["w2ea"] = np.ascontiguousarray(w2p[0:2].reshape(128, 1152)).astype(NP_BF16)
    c["w2eb"] = np.ascontiguousarray(w2p[2:4].reshape(128, 1152)).astype(NP_BF16)

    c["bid1"] = np.tile(np.eye(32, dtype=np.float32), (4, 3)).astype(NP_BF16)
    c["bid2"] = np.tile(np.eye(64, dtype=np.float32), (2, 1)).astype(NP_BF16)
    c["e1x"] = np.repeat(np.eye(4, dtype=np.float32), 32, axis=1)
    c["e2a"] = np.repeat(np.eye(4, dtype=np.float32)[:, 0:2], 64, axis=1)
    c["e2b"] = np.repeat(np.eye(4, dtype=np.float32)[:, 2:4], 64, axis=1)

    c["r1l"] = np.ascontiguousarray((np.asarray(i["r1_w"], np.float32) / 12544.0).T)
    c["r1b"] = np.asarray(i["r1_b"], np.float32).reshape(4, 1)
    c["r2l"] = np.ascontiguousarray((np.asarray(i["r2_w"], np.float32) / 12544.0).T)
    c["r2b"] = np.asarray(i["r2_b"], np.float32).reshape(4, 1)
    c["c1bT"] = np.asarray(i["c1_b"], np.float32) * s1[None, :]
    c["bn1h64"] = h1v.reshape(64, 1)
    c["c2bT"] = np.asarray(i["c2_b"], np.float32) * s2[None, :]
    c["bn2h"] = h2v.reshape(128, 1)
    c["bn0h"] = np.tile(h0v, 4).reshape(128, 1)
    c["fcl"] = np.ascontiguousarray((np.asarray(i["fc_w"], np.float32) / 3136.0).T)
    c["fcb"] = np.asarray(i["fc_b"], np.float32).reshape(2, 1)
    return c


_PROG = None


def pack_xr(x8):
    # x8: [8, 3, 224, 224] float32 (one core's samples)
    out = np.zeros((2, 108, 112, 228), np.float32)
    for g in range(2):
        for kx in range(3):
            cl, ch = max(0, 2 - kx), min(228, 226 - kx + 2)
            # valid C range: [2-kx, 226-kx)
            c0 = 2 - kx if kx < 2 else 0
            c1 = 226 - kx
            xl0 = c0 - 2 + kx  # first x col
            for s in range(4):
                xi = x8[4 * g + s]
                b = 36 * kx + 9 * s
                out[g, b + 0 : b + 3, 1:112, c0:c1] = xi[:, 1:222:2, xl0 : xl0 + (c1 - c0)]
                out[g, b + 3 : b + 6, 0:112, c0:c1] = xi[:, 0:223:2, xl0 : xl0 + (c1 - c0)]
                out[g, b + 6 : b + 9, 0:112, c0:c1] = xi[:, 1:224:2, xl0 : xl0 + (c1 - c0)]
    return out.reshape(2, 108, 112 * 228).astype(NP_FP8)


def kernel(**inputs):
    global _PROG
    if _PROG is None:
        _PROG = build_program()
    nc = _PROG
    c = prep_consts(inputs)
    x = np.asarray(inputs["x"], np.float32)
    in_maps = []
    for core in range(8):
        m = dict(c)
        xp = pack_xr(x[core * 8 : core * 8 + 8])
        m["xrg0"] = np.ascontiguousarray(xp[0])
        m["xrg1"] = np.ascontiguousarray(xp[1])
        in_maps.append(m)
    res = run_bass_kernel_spmd(nc, in_maps, core_ids=list(range(8)))
    out = np.concatenate([r["out"].T for r in res.results], axis=0)
    return out.astype(np.float32)
